# revision 32
# baseline (speedup 1.0000x reference)
"""Trainium2 Bass kernel for nn_DynamicAttentionModel.

Model math (see reference):
    z          = seed_emb[seeds]                          [B, Z]
    h          = relu(z @ hw1 + hb1)                      [B, H]
    coeffs_div = softmax(h @ hw2 + hb2, axis=1)           [B, NB]
    coeffs     = softmax(static_coeffs, 1) + coeffs_div   [B, NB]
    q/k/v      = einsum('bi,bj,ijk->bk', coeffs, feat, W{q,k,v})
    scores     = <q,k>/sqrt(D); attn = softmax over a SINGLE element == 1.0
    pooled     = attn * v == v
    logits     = pooled @ cw + cb

Because attn_map is softmax over one element it is identically 1.0, so q and k
never influence the output: logits depends only on the v-projection.

    logits[b] = sum_i coeffs[b,i] * (feat[b] @ Wv_i @ cw) + cb
    attn_map  = ones([B,1,1])

Sharding: one basis i per NeuronCore (NB == 8 == n_cores). Each core streams
its own Wv_i (fp16 on host; 8.4 MB — the memory-roofline term), computes
    A_i = (feat @ Wv_i) @ cw        fp16 matmul, fp32 accum, PE-transpose +
                                    fp32 classifier, all on device
    e   = exp(relu(z@hw1+hb1) @ hw2 + hb2)     (device, fp32, replicated;
                                    logit range is ~[-3,3] so the
                                    max-subtraction is unnecessary)
Host combine (the unshard step):
    coeffs = softmax(static_coeffs) + e / e.sum(1)
    logits = sum_i coeffs[:, i:i+1] * A_i + cb

Performance structure (from trace iteration):
  * every dma_start costs ~0.6-1.2us of its HWDGE ring's sequencer, and a
    ring holds only 2 in-flight DMAs -> all small f32 params are packed on
    the host into ONE [128, 490] buffer moved by a single DMA;
  * the 16 W chunks are issued up-front, alternating between the SP and ACT
    rings, before any ACT compute exists on the ACT stream;
  * the classifier for block nt-1 is emitted after block nt's Y matmuls, so
    its PE stalls never delay Y work; the hypernetwork sits between block 0
    and block 1 where PE is DMA-waiting anyway.
"""

import os
from contextlib import ExitStack

import numpy as np

import concourse.bass as bass
import concourse.tile as tile
from concourse import bacc, mybir
from concourse.bass_utils import run_bass_kernel_spmd

B, D, NB, Z, POOL, C = 16, 2048, 8, 64, 2048, 10
H = 256
P = 128
NCORES = 8
f32 = mybir.dt.float32
f16 = mybir.dt.float16

W_DTYPE = os.environ.get("KERNEL_W_DTYPE", "f16")

N_TILE = 512                    # output column block (one PSUM bank)
N_BLOCKS = D // N_TILE          # 4
K_TILES = D // P                # 16
JQ = 4                          # j-tiles of 128 rows per W DMA chunk
NCHUNK = N_BLOCKS * (K_TILES // JQ)

# const-pack column offsets (fp32 columns)
_OFF_CW = 0                        # [128, 160]
_OFF_HW2 = 160                     # [128, 16]
_OFF_HB1 = 176                     # [128, 2]
_OFF_ZT = 178                      # [64, 16]
_OFF_HW1 = 194                     # [64, 256]
_OFF_ID = 450                      # [16, 16]
_OFF_HB2 = 466                     # [1, 8]
_OFF_ONES = 474                    # [1, 16]
_PACK_COLS = 490


def _build(w_dt):
    nc = bacc.Bacc(
        "TRN2", target_bir_lowering=False, debug=False, num_devices=NCORES
    )

    # wv pre-tiled on host: [nt, jq, p, jj*N_TILE] so each DMA chunk is
    # [128, JQ*N_TILE] with JQ KB (fp16) contiguous per partition.
    wv = nc.dram_tensor(
        "wv", [N_BLOCKS, K_TILES // JQ, P, JQ * N_TILE], w_dt,
        kind="ExternalInput",
    ).ap()
    featT = nc.dram_tensor("featT", [P, K_TILES * B], w_dt, kind="ExternalInput").ap()
    cpack = nc.dram_tensor("cpack", [P, _PACK_COLS], f32, kind="ExternalInput").ap()
    out = nc.dram_tensor("out", [B, C], f32, kind="ExternalOutput").ap()
    out_e = nc.dram_tensor("out_e", [B, NB], f32, kind="ExternalOutput").ap()

    with tile.TileContext(nc) as tc, ExitStack() as ctx:
        const = ctx.enter_context(tc.tile_pool(name="const", bufs=1))
        wpool = ctx.enter_context(tc.tile_pool(name="wpool", bufs=NCHUNK))
        sb = ctx.enter_context(tc.tile_pool(name="sb", bufs=2))
        sb4 = ctx.enter_context(tc.tile_pool(name="sb4", bufs=4))
        ps_y = ctx.enter_context(tc.tile_pool(name="ps_y", bufs=2, space="PSUM"))
        ps_t = ctx.enter_context(tc.tile_pool(name="ps_t", bufs=4, space="PSUM"))
        ps_o = ctx.enter_context(tc.tile_pool(name="ps_o", bufs=1, space="PSUM"))

        # featT rides first on the SP ring (needed by the first matmul);
        # the f32 const pack is one DMA on the ACT ring.
        feat_sb = const.tile([P, K_TILES, B], w_dt)
        nc.sync.dma_start(feat_sb[:], featT.rearrange("p (t b) -> p t b", t=K_TILES))
        cp = const.tile([P, _PACK_COLS], f32)
        nc.scalar.dma_start(cp[:], cpack)

        cw_sb = cp[:, _OFF_CW:_OFF_CW + K_TILES * C].rearrange(
            "p (t c) -> p t c", t=K_TILES
        )
        hw2_sb = cp[:, _OFF_HW2:_OFF_HW2 + (H // P) * NB].rearrange(
            "p (t n) -> p t n", t=H // P
        )
        hb1_sb = cp[:, _OFF_HB1:_OFF_HB1 + H // P]
        zT_sb = cp[0:Z, _OFF_ZT:_OFF_ZT + B]
        hw1_sb = cp[0:Z, _OFF_HW1:_OFF_HW1 + H]
        ident_sb = cp[0:B, _OFF_ID:_OFF_ID + B]
        hb2_sb = cp[0:1, _OFF_HB2:_OFF_HB2 + NB]
        ones_sb = cp[0:1, _OFF_ONES:_OFF_ONES + B]

        # ---- all W chunk DMAs issued up-front, alternating rings ----------
        wts = []
        for g in range(NCHUNK):
            nt, jq = divmod(g, K_TILES // JQ)
            wt = wpool.tile([P, JQ * N_TILE], w_dt, name="wt")
            (nc.sync if g % 2 == 0 else nc.scalar).dma_start(wt[:], wv[nt, jq])
            wts.append(wt)

        po = ps_o.tile([B, C], f32)

        def y_block(nt, inject=None):
            py = ps_y.tile([B, N_TILE], f32, name="py")
            for jq in range(K_TILES // JQ):
                wt = wts[nt * (K_TILES // JQ) + jq]
                for jj in range(JQ):
                    jt = jq * JQ + jj
                    nc.tensor.matmul(
                        py[:], lhsT=feat_sb[:, jt, :],
                        rhs=wt[:, jj * N_TILE:(jj + 1) * N_TILE],
                        start=(jt == 0), stop=(jt == K_TILES - 1),
                    )
                if inject is not None and jq == 1:
                    inject()
                    inject = None
            return py

        def copy_split(dst, src, ss):
            if ss % 2 == 0:
                nc.scalar.copy(dst, src)
            else:
                nc.vector.tensor_copy(dst, src)

        def classifier(nt, py):
            # transpose Y columns (PE, back-to-back), ACT/DVE copies chase
            # them, then the four classifier matmuls accumulate into po
            ysb = sb.tile([B, N_TILE], f32, name="ysb")
            half = N_TILE // 2
            nc.scalar.copy(ysb[:, 0:half], py[:, 0:half])
            nc.vector.tensor_copy(ysb[:, half:], py[:, half:])
            yts = []
            for ss in range(N_TILE // P):
                pt = ps_t.tile([P, B], f32, name="pt")
                nc.tensor.transpose(pt[:], ysb[:, ss * P:(ss + 1) * P], ident_sb)
                yt = sb4.tile([P, B], f32, name="yt")
                # split the PSUM->SBUF copies across ACT and DVE so the
                # final chain isn't serialized on one engine
                copy_split(yt[:], pt[:], ss)
                yts.append(yt)
            for ss in range(N_TILE // P):
                kt = nt * (N_TILE // P) + ss
                nc.tensor.matmul(
                    po[:], lhsT=yts[ss][:], rhs=cw_sb[:, kt, :],
                    start=(kt == 0), stop=(kt == K_TILES - 1),
                )

        def hypernet():
            # e = exp(relu(z@hw1+hb1) @ hw2 + hb2); normalization on host
            hrT = []
            for t in range(H // P):
                ph = ps_t.tile([P, B], f32, name="pt")
                nc.tensor.matmul(
                    ph[:], lhsT=hw1_sb[:, t * P:(t + 1) * P], rhs=zT_sb,
                    start=True, stop=True,
                )
                hr = sb.tile([P, B], f32, name=f"hr{t}")
                nc.scalar.activation(
                    hr[:], ph[:], mybir.ActivationFunctionType.Relu,
                    bias=hb1_sb[:, t:t + 1],
                )
                hrT.append(hr)
            pl = ps_t.tile([B, NB], f32, name="pt")
            nc.tensor.matmul(pl[:], lhsT=hrT[0][:], rhs=hw2_sb[:, 0, :],
                             start=True, stop=False)
            nc.tensor.matmul(pl[:], lhsT=hrT[1][:], rhs=hw2_sb[:, 1, :],
                             start=False, stop=False)
            # rank-1 trick: broadcast-add hb2 over the batch rows
            nc.tensor.matmul(pl[:], lhsT=ones_sb, rhs=hb2_sb,
                             start=False, stop=True)
            e_sb = sb.tile([B, NB], f32, name="e_sb")
            nc.scalar.activation(e_sb[:], pl[:], mybir.ActivationFunctionType.Exp)
            nc.scalar.dma_start(out_e, e_sb[:])

        pys = {}
        pys[0] = y_block(0)
        hypernet()                      # fills the nt=1 DMA-wait gap
        pys[1] = y_block(1)
        classifier(0, pys[0])
        pys[2] = y_block(2)
        classifier(1, pys[1])
        pys[3] = y_block(3)
        classifier(2, pys[2])
        classifier(3, pys[3])

        osb = sb.tile([B, C], f32, name="osb")
        nc.scalar.copy(osb[:], po[:])
        nc.sync.dma_start(out, osb[:])

    nc.compile()
    return nc


_CACHE = {}


def _get_program():
    if W_DTYPE not in _CACHE:
        _CACHE[W_DTYPE] = _build(f16 if W_DTYPE == "f16" else f32)
    return _CACHE[W_DTYPE]


def _np_softmax(x, axis):
    x = x - x.max(axis=axis, keepdims=True)
    e = np.exp(x)
    return e / e.sum(axis=axis, keepdims=True)


def _tile_w(w, np_wdt):
    # [D, D] -> [nt, jq, p, jj*N]  with  [p, jj*N+n] = w[jq*JQ*P + jj*P + p,
    #                                                    nt*N_TILE + n]
    t = w.reshape(K_TILES // JQ, JQ, P, N_BLOCKS, N_TILE)
    t = t.transpose(3, 0, 2, 1, 4).reshape(
        N_BLOCKS, K_TILES // JQ, P, JQ * N_TILE
    )
    return np.ascontiguousarray(t.astype(np_wdt))


def _tile_rows(x, np_dt):
    # [D, M] -> [p, t*M] with [p, t*M+m] = x[t*P+p, m]
    n, m = x.shape
    t = x.reshape(n // P, P, m).transpose(1, 0, 2).reshape(P, (n // P) * m)
    return np.ascontiguousarray(t.astype(np_dt))


def _const_pack(cw, hw2, hb1, z, hw1, hb2):
    pk = np.zeros((P, _PACK_COLS), np.float32)
    pk[:, _OFF_CW:_OFF_CW + K_TILES * C] = _tile_rows(cw, np.float32)
    pk[:, _OFF_HW2:_OFF_HW2 + (H // P) * NB] = _tile_rows(hw2, np.float32)
    pk[:, _OFF_HB1:_OFF_HB1 + H // P] = hb1.reshape(H // P, P).T
    pk[0:Z, _OFF_ZT:_OFF_ZT + B] = z.T
    pk[0:Z, _OFF_HW1:_OFF_HW1 + H] = hw1
    pk[0:B, _OFF_ID:_OFF_ID + B] = np.eye(B, dtype=np.float32)
    pk[0, _OFF_HB2:_OFF_HB2 + NB] = hb2.reshape(NB)
    pk[0, _OFF_ONES:_OFF_ONES + B] = 1.0
    return pk


def kernel(features, seeds, seed_emb, static_coeffs, hw1, hb1, hw2, hb2,
           wq, wk, wv, cw, cb, _run_kwargs=None, _results_out=None):
    features = np.asarray(features, dtype=np.float32)
    seeds = np.asarray(seeds).astype(np.int64)
    seed_emb = np.asarray(seed_emb, dtype=np.float32)
    static_coeffs = np.asarray(static_coeffs, dtype=np.float32)
    hw1 = np.asarray(hw1, dtype=np.float32)
    hb1 = np.asarray(hb1, dtype=np.float32)
    hw2 = np.asarray(hw2, dtype=np.float32)
    hb2 = np.asarray(hb2, dtype=np.float32)
    wv = np.asarray(wv, dtype=np.float32)
    cw = np.asarray(cw, dtype=np.float32)
    cb = np.asarray(cb, dtype=np.float32)

    np_wdt = np.float16 if W_DTYPE == "f16" else np.float32
    z = seed_emb[seeds]
    shared = {
        "featT": _tile_rows(features.T, np_wdt),
        "cpack": _const_pack(cw, hw2, hb1, z, hw1, hb2),
    }
    in_maps = [{**shared, "wv": _tile_w(wv[i], np_wdt)} for i in range(NCORES)]

    nc = _get_program()
    res = run_bass_kernel_spmd(
        nc, in_maps, core_ids=list(range(NCORES)), **(_run_kwargs or {})
    )
    if _results_out is not None:
        _results_out.append(res)

    e = res.results[0]["out_e"]
    coeffs = _np_softmax(static_coeffs, 1) + e / e.sum(axis=1, keepdims=True)
    logits = np.zeros((B, C), np.float32)
    for i in range(NCORES):
        logits += coeffs[:, i:i + 1] * res.results[i]["out"]
    logits += cb
    attn_map = np.ones((B, 1, 1), np.float32)
    return logits, attn_map


# revision 35
# speedup vs baseline: 1.0208x; 1.0208x over previous
"""Trainium2 Bass kernel for nn_DynamicAttentionModel.

Model math (see reference):
    z          = seed_emb[seeds]                          [B, Z]
    h          = relu(z @ hw1 + hb1)                      [B, H]
    coeffs_div = softmax(h @ hw2 + hb2, axis=1)           [B, NB]
    coeffs     = softmax(static_coeffs, 1) + coeffs_div   [B, NB]
    q/k/v      = einsum('bi,bj,ijk->bk', coeffs, feat, W{q,k,v})
    scores     = <q,k>/sqrt(D); attn = softmax over a SINGLE element == 1.0
    pooled     = attn * v == v
    logits     = pooled @ cw + cb

Because attn_map is softmax over one element it is identically 1.0, so q and k
never influence the output: logits depends only on the v-projection.

    logits[b] = sum_i coeffs[b,i] * (feat[b] @ Wv_i @ cw) + cb
    attn_map  = ones([B,1,1])

Sharding: one basis i per NeuronCore (NB == 8 == n_cores). Each core streams
its own Wv_i (fp16 on host; 8.4 MB — the memory-roofline term), computes
    A_i = (feat @ Wv_i) @ cw        fp16 matmul, fp32 accum, PE-transpose +
                                    fp32 classifier, all on device
    e   = exp(relu(z@hw1+hb1) @ hw2 + hb2)     (device, fp32, replicated;
                                    logit range is ~[-3,3] so the
                                    max-subtraction is unnecessary)
Host combine (the unshard step):
    coeffs = softmax(static_coeffs) + e / e.sum(1)
    logits = sum_i coeffs[:, i:i+1] * A_i + cb

Performance structure (from trace iteration):
  * every dma_start costs ~0.6-1.2us of its HWDGE ring's sequencer, and a
    ring holds only 2 in-flight DMAs -> all small f32 params are packed on
    the host into ONE [128, 490] buffer moved by a single DMA;
  * the 16 W chunks are issued up-front, alternating between the SP and ACT
    rings, before any ACT compute exists on the ACT stream;
  * the classifier for block nt-1 is emitted after block nt's Y matmuls, so
    its PE stalls never delay Y work; the hypernetwork sits between block 0
    and block 1 where PE is DMA-waiting anyway.
"""

import os
from contextlib import ExitStack

import numpy as np

import concourse.bass as bass
import concourse.tile as tile
from concourse import bacc, mybir
from concourse.bass_utils import run_bass_kernel_spmd

B, D, NB, Z, POOL, C = 16, 2048, 8, 64, 2048, 10
H = 256
P = 128
NCORES = 8
f32 = mybir.dt.float32
f16 = mybir.dt.float16

W_DTYPE = os.environ.get("KERNEL_W_DTYPE", "f16")

N_TILE = 512                    # output column block (one PSUM bank)
N_BLOCKS = D // N_TILE          # 4
K_TILES = D // P                # 16
JQ = 4                          # j-tiles of 128 rows per W DMA chunk
NCHUNK = N_BLOCKS * (K_TILES // JQ)

# const-pack column offsets (fp32 columns)
_OFF_CW = 0                        # [128, 160]
_OFF_HW2 = 160                     # [128, 16]
_OFF_HB1 = 176                     # [128, 2]
_OFF_ZT = 178                      # [64, 16]
_OFF_HW1 = 194                     # [64, 256]
_OFF_ID = 450                      # [16, 16]
_OFF_HB2 = 466                     # [1, 8]
_OFF_ONES = 474                    # [1, 16]
_PACK_COLS = 490


def _build(w_dt):
    nc = bacc.Bacc(
        "TRN2", target_bir_lowering=False, debug=False, num_devices=NCORES
    )

    # wv pre-tiled on host: [nt, jq, p, jj*N_TILE] so each DMA chunk is
    # [128, JQ*N_TILE] with JQ KB (fp16) contiguous per partition.
    wv = nc.dram_tensor(
        "wv", [N_BLOCKS, K_TILES // JQ, P, JQ * N_TILE], w_dt,
        kind="ExternalInput",
    ).ap()
    featT = nc.dram_tensor("featT", [P, K_TILES * B], w_dt, kind="ExternalInput").ap()
    cpack = nc.dram_tensor("cpack", [P, _PACK_COLS], f32, kind="ExternalInput").ap()
    out = nc.dram_tensor("out", [B, C], f32, kind="ExternalOutput").ap()
    out_e = nc.dram_tensor("out_e", [B, NB], f32, kind="ExternalOutput").ap()

    with tile.TileContext(nc) as tc, ExitStack() as ctx:
        const = ctx.enter_context(tc.tile_pool(name="const", bufs=1))
        wpool = ctx.enter_context(tc.tile_pool(name="wpool", bufs=NCHUNK))
        sb = ctx.enter_context(tc.tile_pool(name="sb", bufs=2))
        sb4 = ctx.enter_context(tc.tile_pool(name="sb4", bufs=4))
        ps_y = ctx.enter_context(tc.tile_pool(name="ps_y", bufs=2, space="PSUM"))
        ps_t = ctx.enter_context(tc.tile_pool(name="ps_t", bufs=4, space="PSUM"))
        ps_o = ctx.enter_context(tc.tile_pool(name="ps_o", bufs=1, space="PSUM"))

        # featT rides first on the SP ring (needed by the first matmul);
        # the f32 const pack is one DMA on the ACT ring.
        feat_sb = const.tile([P, K_TILES, B], w_dt)
        nc.sync.dma_start(feat_sb[:], featT.rearrange("p (t b) -> p t b", t=K_TILES))
        cp = const.tile([P, _PACK_COLS], f32)
        nc.scalar.dma_start(cp[:], cpack)

        cw_sb = cp[:, _OFF_CW:_OFF_CW + K_TILES * C].rearrange(
            "p (t c) -> p t c", t=K_TILES
        )
        hw2_sb = cp[:, _OFF_HW2:_OFF_HW2 + (H // P) * NB].rearrange(
            "p (t n) -> p t n", t=H // P
        )
        hb1_sb = cp[:, _OFF_HB1:_OFF_HB1 + H // P]
        zT_sb = cp[0:Z, _OFF_ZT:_OFF_ZT + B]
        hw1_sb = cp[0:Z, _OFF_HW1:_OFF_HW1 + H]
        ident_sb = cp[0:B, _OFF_ID:_OFF_ID + B]
        hb2_sb = cp[0:1, _OFF_HB2:_OFF_HB2 + NB]
        ones_sb = cp[0:1, _OFF_ONES:_OFF_ONES + B]

        # ---- all W chunk DMAs issued up-front, alternating rings ----------
        wts = []
        for g in range(NCHUNK):
            nt, jq = divmod(g, K_TILES // JQ)
            wt = wpool.tile([P, JQ * N_TILE], w_dt, name="wt")
            (nc.sync if g % 2 == 0 else nc.scalar).dma_start(wt[:], wv[nt, jq])
            wts.append(wt)

        po = ps_o.tile([B, C], f32)

        def y_block(nt, inject=None):
            py = ps_y.tile([B, N_TILE], f32, name="py")
            for jq in range(K_TILES // JQ):
                wt = wts[nt * (K_TILES // JQ) + jq]
                for jj in range(JQ):
                    jt = jq * JQ + jj
                    nc.tensor.matmul(
                        py[:], lhsT=feat_sb[:, jt, :],
                        rhs=wt[:, jj * N_TILE:(jj + 1) * N_TILE],
                        start=(jt == 0), stop=(jt == K_TILES - 1),
                    )
                if inject is not None and jq == 1:
                    inject()
                    inject = None
            return py

        def copy_split(dst, src, ss):
            if ss % 2 == 0:
                nc.scalar.copy(dst, src)
            else:
                nc.vector.tensor_copy(dst, src)

        def classifier(nt, py):
            # transpose Y columns (PE, back-to-back), ACT/DVE copies chase
            # them, then the four classifier matmuls accumulate into po
            ysb = sb.tile([B, N_TILE], f32, name="ysb")
            half = N_TILE // 2
            nc.scalar.copy(ysb[:, 0:half], py[:, 0:half])
            nc.vector.tensor_copy(ysb[:, half:], py[:, half:])
            yts = []
            for ss in range(N_TILE // P):
                pt = ps_t.tile([P, B], f32, name="pt")
                nc.tensor.transpose(pt[:], ysb[:, ss * P:(ss + 1) * P], ident_sb)
                yt = sb4.tile([P, B], f32, name="yt")
                # split the PSUM->SBUF copies across ACT and DVE so the
                # final chain isn't serialized on one engine
                copy_split(yt[:], pt[:], ss)
                yts.append(yt)
            for ss in range(N_TILE // P):
                kt = nt * (N_TILE // P) + ss
                nc.tensor.matmul(
                    po[:], lhsT=yts[ss][:], rhs=cw_sb[:, kt, :],
                    start=(kt == 0), stop=(kt == K_TILES - 1),
                )

        def hypernet():
            # e = exp(relu(z@hw1+hb1) @ hw2 + hb2); normalization on host
            hrT = []
            for t in range(H // P):
                ph = ps_t.tile([P, B], f32, name="pt")
                nc.tensor.matmul(
                    ph[:], lhsT=hw1_sb[:, t * P:(t + 1) * P], rhs=zT_sb,
                    start=True, stop=True,
                )
                hr = sb.tile([P, B], f32, name=f"hr{t}")
                nc.scalar.activation(
                    hr[:], ph[:], mybir.ActivationFunctionType.Relu,
                    bias=hb1_sb[:, t:t + 1],
                )
                hrT.append(hr)
            pl = ps_t.tile([B, NB], f32, name="pt")
            nc.tensor.matmul(pl[:], lhsT=hrT[0][:], rhs=hw2_sb[:, 0, :],
                             start=True, stop=False)
            nc.tensor.matmul(pl[:], lhsT=hrT[1][:], rhs=hw2_sb[:, 1, :],
                             start=False, stop=False)
            # rank-1 trick: broadcast-add hb2 over the batch rows
            nc.tensor.matmul(pl[:], lhsT=ones_sb, rhs=hb2_sb,
                             start=False, stop=True)
            e_sb = sb.tile([B, NB], f32, name="e_sb")
            nc.scalar.activation(e_sb[:], pl[:], mybir.ActivationFunctionType.Exp)
            nc.scalar.dma_start(out_e, e_sb[:])

        pys = {}
        pys[0] = y_block(0)
        hypernet()                      # fills the nt=1 DMA-wait gap
        pys[1] = y_block(1)
        classifier(0, pys[0])
        pys[2] = y_block(2)
        classifier(1, pys[1])
        # classifier(2) sits between block 3's jq1 and jq2, where even a
        # fair-arbitration core still has ~2us of DMA wait left
        pys[3] = y_block(3, inject=lambda: classifier(2, pys[2]))
        classifier(3, pys[3])

        osb = sb.tile([B, C], f32, name="osb")
        nc.scalar.copy(osb[:], po[:])
        nc.sync.dma_start(out, osb[:])

    nc.compile()
    return nc


_CACHE = {}


def _get_program():
    if W_DTYPE not in _CACHE:
        _CACHE[W_DTYPE] = _build(f16 if W_DTYPE == "f16" else f32)
    return _CACHE[W_DTYPE]


def _np_softmax(x, axis):
    x = x - x.max(axis=axis, keepdims=True)
    e = np.exp(x)
    return e / e.sum(axis=axis, keepdims=True)


def _tile_w(w, np_wdt):
    # [D, D] -> [nt, jq, p, jj*N]  with  [p, jj*N+n] = w[jq*JQ*P + jj*P + p,
    #                                                    nt*N_TILE + n]
    t = w.reshape(K_TILES // JQ, JQ, P, N_BLOCKS, N_TILE)
    t = t.transpose(3, 0, 2, 1, 4).reshape(
        N_BLOCKS, K_TILES // JQ, P, JQ * N_TILE
    )
    return np.ascontiguousarray(t.astype(np_wdt))


def _tile_rows(x, np_dt):
    # [D, M] -> [p, t*M] with [p, t*M+m] = x[t*P+p, m]
    n, m = x.shape
    t = x.reshape(n // P, P, m).transpose(1, 0, 2).reshape(P, (n // P) * m)
    return np.ascontiguousarray(t.astype(np_dt))


def _const_pack(cw, hw2, hb1, z, hw1, hb2):
    pk = np.zeros((P, _PACK_COLS), np.float32)
    pk[:, _OFF_CW:_OFF_CW + K_TILES * C] = _tile_rows(cw, np.float32)
    pk[:, _OFF_HW2:_OFF_HW2 + (H // P) * NB] = _tile_rows(hw2, np.float32)
    pk[:, _OFF_HB1:_OFF_HB1 + H // P] = hb1.reshape(H // P, P).T
    pk[0:Z, _OFF_ZT:_OFF_ZT + B] = z.T
    pk[0:Z, _OFF_HW1:_OFF_HW1 + H] = hw1
    pk[0:B, _OFF_ID:_OFF_ID + B] = np.eye(B, dtype=np.float32)
    pk[0, _OFF_HB2:_OFF_HB2 + NB] = hb2.reshape(NB)
    pk[0, _OFF_ONES:_OFF_ONES + B] = 1.0
    return pk


def kernel(features, seeds, seed_emb, static_coeffs, hw1, hb1, hw2, hb2,
           wq, wk, wv, cw, cb, _run_kwargs=None, _results_out=None):
    features = np.asarray(features, dtype=np.float32)
    seeds = np.asarray(seeds).astype(np.int64)
    seed_emb = np.asarray(seed_emb, dtype=np.float32)
    static_coeffs = np.asarray(static_coeffs, dtype=np.float32)
    hw1 = np.asarray(hw1, dtype=np.float32)
    hb1 = np.asarray(hb1, dtype=np.float32)
    hw2 = np.asarray(hw2, dtype=np.float32)
    hb2 = np.asarray(hb2, dtype=np.float32)
    wv = np.asarray(wv, dtype=np.float32)
    cw = np.asarray(cw, dtype=np.float32)
    cb = np.asarray(cb, dtype=np.float32)

    np_wdt = np.float16 if W_DTYPE == "f16" else np.float32
    z = seed_emb[seeds]
    shared = {
        "featT": _tile_rows(features.T, np_wdt),
        "cpack": _const_pack(cw, hw2, hb1, z, hw1, hb2),
    }
    in_maps = [{**shared, "wv": _tile_w(wv[i], np_wdt)} for i in range(NCORES)]

    nc = _get_program()
    res = run_bass_kernel_spmd(
        nc, in_maps, core_ids=list(range(NCORES)), **(_run_kwargs or {})
    )
    if _results_out is not None:
        _results_out.append(res)

    e = res.results[0]["out_e"]
    coeffs = _np_softmax(static_coeffs, 1) + e / e.sum(axis=1, keepdims=True)
    logits = np.zeros((B, C), np.float32)
    for i in range(NCORES):
        logits += coeffs[:, i:i + 1] * res.results[i]["out"]
    logits += cb
    attn_map = np.ones((B, 1, 1), np.float32)
    return logits, attn_map


# revision 39
# speedup vs baseline: 1.1169x; 1.0942x over previous
"""Trainium2 Bass kernel for nn_DynamicAttentionModel.

Model math (see reference):
    z          = seed_emb[seeds]                          [B, Z]
    h          = relu(z @ hw1 + hb1)                      [B, H]
    coeffs_div = softmax(h @ hw2 + hb2, axis=1)           [B, NB]
    coeffs     = softmax(static_coeffs, 1) + coeffs_div   [B, NB]
    q/k/v      = einsum('bi,bj,ijk->bk', coeffs, feat, W{q,k,v})
    scores     = <q,k>/sqrt(D); attn = softmax over a SINGLE element == 1.0
    pooled     = attn * v == v
    logits     = pooled @ cw + cb

Because attn_map is softmax over one element it is identically 1.0, so q and k
never influence the output: logits depends only on the v-projection.

    logits[b] = sum_i coeffs[b,i] * (feat[b] @ Wv_i @ cw) + cb
    attn_map  = ones([B,1,1])

Sharding: one basis i per NeuronCore (NB == 8 == n_cores). Each core streams
its own Wv_i (fp16 on host; 8.4 MB — the memory-roofline term), computes
    A_i = (feat @ Wv_i) @ cw        fp16 matmul, fp32 accum, PE-transpose +
                                    fp32 classifier, all on device
    e   = exp(relu(z@hw1+hb1) @ hw2 + hb2)     (device, fp32, replicated;
                                    logit range is ~[-3,3] so the
                                    max-subtraction is unnecessary)
Host combine (the unshard step):
    coeffs = softmax(static_coeffs) + e / e.sum(1)
    logits = sum_i coeffs[:, i:i+1] * A_i + cb

Performance structure (from trace iteration):
  * every dma_start costs ~0.6-1.2us of its HWDGE ring's sequencer, and a
    ring holds only 2 in-flight DMAs -> all small f32 params are packed on
    the host into ONE [128, 490] buffer moved by a single DMA;
  * the 16 W chunks are issued up-front, alternating between the SP and ACT
    rings, before any ACT compute exists on the ACT stream;
  * the classifier for block nt-1 is emitted after block nt's Y matmuls, so
    its PE stalls never delay Y work; the hypernetwork sits between block 0
    and block 1 where PE is DMA-waiting anyway.
"""

import os
from contextlib import ExitStack

import numpy as np

import concourse.bass as bass
import concourse.tile as tile
from concourse import bacc, mybir
from concourse.bass_utils import run_bass_kernel_spmd

B, D, NB, Z, POOL, C = 16, 2048, 8, 64, 2048, 10
H = 256
P = 128
NCORES = 8
f32 = mybir.dt.float32
f16 = mybir.dt.float16

W_DTYPE = os.environ.get("KERNEL_W_DTYPE", "f16")

N_TILE = 512                    # output column block (one PSUM bank)
N_BLOCKS = D // N_TILE          # 4
K_TILES = D // P                # 16
JQ = 4                          # j-tiles of 128 rows per W DMA chunk
NCHUNK = N_BLOCKS * (K_TILES // JQ)

# const-pack column offsets (fp32 columns)
_OFF_CW = 0                        # [128, 160]
_OFF_HW2 = 160                     # [128, 16]
_OFF_HB1 = 176                     # [128, 2]
_OFF_ZT = 178                      # [64, 16]
_OFF_HW1 = 194                     # [64, 256]
_OFF_ID = 450                      # [16, 16]
_OFF_HB2 = 466                     # [1, 8]
_OFF_ONES = 474                    # [1, 16]
_PACK_COLS = 490


def _build(w_dt):
    nc = bacc.Bacc(
        "TRN2", target_bir_lowering=False, debug=False, num_devices=NCORES
    )

    # wv pre-tiled on host: [nt, jq, p, jj*N_TILE] so each DMA chunk is
    # [128, JQ*N_TILE] with JQ KB (fp16) contiguous per partition.
    wv = nc.dram_tensor(
        "wv", [N_BLOCKS, K_TILES // JQ, P, JQ * N_TILE], w_dt,
        kind="ExternalInput",
    ).ap()
    # first two W chunks carry piggyback payloads so no separate small DMAs
    # delay the ring starts: chunk0 += featT (fp16), chunk1 += const pack
    # (f32 viewed as fp16 pairs, bitcast back on SBUF)
    w0 = nc.dram_tensor("w0", [P, JQ * N_TILE + K_TILES * B], w_dt,
                        kind="ExternalInput").ap()
    w1 = nc.dram_tensor("w1", [P, JQ * N_TILE + 2 * _PACK_COLS], w_dt,
                        kind="ExternalInput").ap()
    out = nc.dram_tensor("out", [B, C], f32, kind="ExternalOutput").ap()
    out_e = nc.dram_tensor("out_e", [B, NB], f32, kind="ExternalOutput").ap()

    with tile.TileContext(nc) as tc, ExitStack() as ctx:
        const = ctx.enter_context(tc.tile_pool(name="const", bufs=1))
        wpool = ctx.enter_context(tc.tile_pool(name="wpool", bufs=NCHUNK))
        sb = ctx.enter_context(tc.tile_pool(name="sb", bufs=2))
        sb4 = ctx.enter_context(tc.tile_pool(name="sb4", bufs=4))
        ps_y = ctx.enter_context(tc.tile_pool(name="ps_y", bufs=2, space="PSUM"))
        ps_t = ctx.enter_context(tc.tile_pool(name="ps_t", bufs=4, space="PSUM"))
        ps_o = ctx.enter_context(tc.tile_pool(name="ps_o", bufs=1, space="PSUM"))

        # first two W chunks (with piggyback payloads) open the two rings
        wt0 = const.tile([P, JQ * N_TILE + K_TILES * B], w_dt, name="wt0")
        nc.sync.dma_start(wt0[:], w0)
        wt1 = const.tile([P, JQ * N_TILE + 2 * _PACK_COLS], w_dt, name="wt1")
        nc.scalar.dma_start(wt1[:], w1)

        feat_sb = wt0[:, JQ * N_TILE:].rearrange("p (t b) -> p t b", t=K_TILES)
        cp = wt1[:, JQ * N_TILE:].bitcast(f32)

        cw_sb = cp[:, _OFF_CW:_OFF_CW + K_TILES * C].rearrange(
            "p (t c) -> p t c", t=K_TILES
        )
        hw2_sb = cp[:, _OFF_HW2:_OFF_HW2 + (H // P) * NB].rearrange(
            "p (t n) -> p t n", t=H // P
        )
        hb1_sb = cp[:, _OFF_HB1:_OFF_HB1 + H // P]
        zT_sb = cp[0:Z, _OFF_ZT:_OFF_ZT + B]
        hw1_sb = cp[0:Z, _OFF_HW1:_OFF_HW1 + H]
        ident_sb = cp[0:B, _OFF_ID:_OFF_ID + B]
        hb2_sb = cp[0:1, _OFF_HB2:_OFF_HB2 + NB]
        ones_sb = cp[0:1, _OFF_ONES:_OFF_ONES + B]

        # ---- all remaining W chunk DMAs issued up-front, alternating rings
        wts = [wt0[:, 0:JQ * N_TILE], wt1[:, 0:JQ * N_TILE]]
        for g in range(2, NCHUNK):
            nt, jq = divmod(g, K_TILES // JQ)
            wt = wpool.tile([P, JQ * N_TILE], w_dt, name="wt")
            (nc.sync if g % 2 == 0 else nc.scalar).dma_start(wt[:], wv[nt, jq])
            wts.append(wt[:])

        po = ps_o.tile([B, C], f32)

        def y_block(nt, inject=None):
            py = ps_y.tile([B, N_TILE], f32, name="py")
            for jq in range(K_TILES // JQ):
                wt = wts[nt * (K_TILES // JQ) + jq]
                for jj in range(JQ):
                    jt = jq * JQ + jj
                    nc.tensor.matmul(
                        py[:], lhsT=feat_sb[:, jt, :],
                        rhs=wt[:, jj * N_TILE:(jj + 1) * N_TILE],
                        start=(jt == 0), stop=(jt == K_TILES - 1),
                    )
                if inject is not None and jq == 1:
                    inject()
                    inject = None
            return py

        def copy_split(dst, src, ss):
            if ss % 2 == 0:
                nc.scalar.copy(dst, src)
            else:
                nc.vector.tensor_copy(dst, src)

        def classifier(nt, py):
            # transpose Y columns (PE, back-to-back), ACT/DVE copies chase
            # them, then the four classifier matmuls accumulate into po
            ysb = sb.tile([B, N_TILE], f32, name="ysb")
            half = N_TILE // 2
            nc.scalar.copy(ysb[:, 0:half], py[:, 0:half])
            nc.vector.tensor_copy(ysb[:, half:], py[:, half:])
            yts = []
            for ss in range(N_TILE // P):
                pt = ps_t.tile([P, B], f32, name="pt")
                nc.tensor.transpose(pt[:], ysb[:, ss * P:(ss + 1) * P], ident_sb)
                yt = sb4.tile([P, B], f32, name="yt")
                # split the PSUM->SBUF copies across ACT and DVE so the
                # final chain isn't serialized on one engine
                copy_split(yt[:], pt[:], ss)
                yts.append(yt)
            for ss in range(N_TILE // P):
                kt = nt * (N_TILE // P) + ss
                nc.tensor.matmul(
                    po[:], lhsT=yts[ss][:], rhs=cw_sb[:, kt, :],
                    start=(kt == 0), stop=(kt == K_TILES - 1),
                )

        def hypernet():
            # e = exp(relu(z@hw1+hb1) @ hw2 + hb2); normalization on host
            hrT = []
            for t in range(H // P):
                ph = ps_t.tile([P, B], f32, name="pt")
                nc.tensor.matmul(
                    ph[:], lhsT=hw1_sb[:, t * P:(t + 1) * P], rhs=zT_sb,
                    start=True, stop=True,
                )
                hr = sb.tile([P, B], f32, name=f"hr{t}")
                nc.scalar.activation(
                    hr[:], ph[:], mybir.ActivationFunctionType.Relu,
                    bias=hb1_sb[:, t:t + 1],
                )
                hrT.append(hr)
            pl = ps_t.tile([B, NB], f32, name="pt")
            nc.tensor.matmul(pl[:], lhsT=hrT[0][:], rhs=hw2_sb[:, 0, :],
                             start=True, stop=False)
            nc.tensor.matmul(pl[:], lhsT=hrT[1][:], rhs=hw2_sb[:, 1, :],
                             start=False, stop=False)
            # rank-1 trick: broadcast-add hb2 over the batch rows
            nc.tensor.matmul(pl[:], lhsT=ones_sb, rhs=hb2_sb,
                             start=False, stop=True)
            e_sb = sb.tile([B, NB], f32, name="e_sb")
            nc.scalar.activation(e_sb[:], pl[:], mybir.ActivationFunctionType.Exp)
            nc.scalar.dma_start(out_e, e_sb[:])

        pys = {}
        pys[0] = y_block(0)
        hypernet()                      # fills the nt=1 DMA-wait gap
        pys[1] = y_block(1)
        classifier(0, pys[0])
        pys[2] = y_block(2)
        classifier(1, pys[1])
        # classifier(2) sits between block 3's jq1 and jq2, where even a
        # fair-arbitration core still has ~2us of DMA wait left
        pys[3] = y_block(3, inject=lambda: classifier(2, pys[2]))
        classifier(3, pys[3])

        osb = sb.tile([B, C], f32, name="osb")
        nc.scalar.copy(osb[:], po[:])
        nc.sync.dma_start(out, osb[:])

    nc.compile()
    return nc


_CACHE = {}


def _get_program():
    if W_DTYPE not in _CACHE:
        _CACHE[W_DTYPE] = _build(f16 if W_DTYPE == "f16" else f32)
    return _CACHE[W_DTYPE]


def _np_softmax(x, axis):
    x = x - x.max(axis=axis, keepdims=True)
    e = np.exp(x)
    return e / e.sum(axis=axis, keepdims=True)


def _tile_w(w, np_wdt):
    # [D, D] -> [nt, jq, p, jj*N]  with  [p, jj*N+n] = w[jq*JQ*P + jj*P + p,
    #                                                    nt*N_TILE + n]
    t = w.reshape(K_TILES // JQ, JQ, P, N_BLOCKS, N_TILE)
    t = t.transpose(3, 0, 2, 1, 4).reshape(
        N_BLOCKS, K_TILES // JQ, P, JQ * N_TILE
    )
    return np.ascontiguousarray(t.astype(np_wdt))


def _tile_rows(x, np_dt):
    # [D, M] -> [p, t*M] with [p, t*M+m] = x[t*P+p, m]
    n, m = x.shape
    t = x.reshape(n // P, P, m).transpose(1, 0, 2).reshape(P, (n // P) * m)
    return np.ascontiguousarray(t.astype(np_dt))


def _const_pack(cw, hw2, hb1, z, hw1, hb2):
    pk = np.zeros((P, _PACK_COLS), np.float32)
    pk[:, _OFF_CW:_OFF_CW + K_TILES * C] = _tile_rows(cw, np.float32)
    pk[:, _OFF_HW2:_OFF_HW2 + (H // P) * NB] = _tile_rows(hw2, np.float32)
    pk[:, _OFF_HB1:_OFF_HB1 + H // P] = hb1.reshape(H // P, P).T
    pk[0:Z, _OFF_ZT:_OFF_ZT + B] = z.T
    pk[0:Z, _OFF_HW1:_OFF_HW1 + H] = hw1
    pk[0:B, _OFF_ID:_OFF_ID + B] = np.eye(B, dtype=np.float32)
    pk[0, _OFF_HB2:_OFF_HB2 + NB] = hb2.reshape(NB)
    pk[0, _OFF_ONES:_OFF_ONES + B] = 1.0
    return pk


def kernel(features, seeds, seed_emb, static_coeffs, hw1, hb1, hw2, hb2,
           wq, wk, wv, cw, cb, _run_kwargs=None, _results_out=None):
    features = np.asarray(features, dtype=np.float32)
    seeds = np.asarray(seeds).astype(np.int64)
    seed_emb = np.asarray(seed_emb, dtype=np.float32)
    static_coeffs = np.asarray(static_coeffs, dtype=np.float32)
    hw1 = np.asarray(hw1, dtype=np.float32)
    hb1 = np.asarray(hb1, dtype=np.float32)
    hw2 = np.asarray(hw2, dtype=np.float32)
    hb2 = np.asarray(hb2, dtype=np.float32)
    wv = np.asarray(wv, dtype=np.float32)
    cw = np.asarray(cw, dtype=np.float32)
    cb = np.asarray(cb, dtype=np.float32)

    np_wdt = np.float16 if W_DTYPE == "f16" else np.float32
    z = seed_emb[seeds]
    featT = _tile_rows(features.T, np_wdt)
    cp16 = _const_pack(cw, hw2, hb1, z, hw1, hb2).view(np.float16)
    in_maps = []
    for i in range(NCORES):
        wt = _tile_w(wv[i], np_wdt)
        in_maps.append({
            "wv": wt,
            "w0": np.ascontiguousarray(np.concatenate([wt[0, 0], featT], axis=1)),
            "w1": np.ascontiguousarray(np.concatenate([wt[0, 1], cp16], axis=1)),
        })

    nc = _get_program()
    res = run_bass_kernel_spmd(
        nc, in_maps, core_ids=list(range(NCORES)), **(_run_kwargs or {})
    )
    if _results_out is not None:
        _results_out.append(res)

    e = res.results[0]["out_e"]
    coeffs = _np_softmax(static_coeffs, 1) + e / e.sum(axis=1, keepdims=True)
    logits = np.zeros((B, C), np.float32)
    for i in range(NCORES):
        logits += coeffs[:, i:i + 1] * res.results[i]["out"]
    logits += cb
    attn_map = np.ones((B, 1, 1), np.float32)
    return logits, attn_map
